# revision 2
# baseline (speedup 1.0000x reference)
"""Trainium2 Bass kernel for nn_CrossAttentionFusion (dense_transformer).

Strategy: pure data parallel over 8 NeuronCores (batch 32768 -> 4096/core).
Token-major layout: batch rows on SBUF partitions, 4 tokens x 256 features in
the free dimension.  Dense matmuls are activation-stationary bf16 (fp32 PSUM).
Attention (seq=4, 8 heads x 32) runs on the Vector engine in bf16 with
contiguous access patterns: pair-products at 2x, log-tree adds for the d
reduction (tensor_reduce is capped at 1x), softmax via the tanh identity
e^s=(1+t)/(1-t) so the Scalar engine only ever needs Tanh+Gelu (one ACT table
set - no LUT reloads).  LayerNorm rstd is computed on the Vector engine with
the inverse-sqrt bit trick + 3 Newton steps (no Sqrt table), applied via
tensor_scalar with per-partition [P,1] scale/shift.  Residual stream is bf16;
residual adds are single [P,1024] ops after a Scalar-engine PSUM evacuation.
"""

import contextlib
import ctypes
import math
import os
import sys
import types
from contextlib import ExitStack

import numpy as np

import concourse.bass as bass
import concourse.tile as tile
from concourse import mybir
from concourse.bass_utils import run_bass_kernel_spmd
from concourse.masks import make_identity


def _install_ntff_hook_shim():
    """Provide antenv.axon_hooks if the image lacks it, so trace=True works."""
    try:
        import antenv.axon_hooks  # noqa: F401
        return
    except ImportError:
        pass
    so_path = "/opt/axon/libaxon_pjrt.so"
    hook = None
    if os.path.exists(so_path):
        try:
            lib = ctypes.CDLL(so_path)
            if hasattr(lib, "axon_start_nrt_profile"):
                lib.axon_start_nrt_profile.argtypes = [
                    ctypes.POINTER(ctypes.c_int64), ctypes.c_size_t]
                lib.axon_start_nrt_profile.restype = ctypes.c_int64
                lib.axon_stop_nrt_profile.argtypes = [ctypes.c_char_p]
                lib.axon_stop_nrt_profile.restype = ctypes.c_int64

                @contextlib.contextmanager
                def _hook(output_dir, device_ids):
                    import jax
                    jax.devices()
                    if device_ids:
                        ids = (ctypes.c_int64 * len(device_ids))(*device_ids)
                        rc = lib.axon_start_nrt_profile(ids, len(device_ids))
                    else:
                        rc = lib.axon_start_nrt_profile(None, 0)
                    if rc != 0:
                        raise RuntimeError(f"axon_start_nrt_profile rc={rc}")
                    try:
                        yield
                    finally:
                        n = lib.axon_stop_nrt_profile(str(output_dir).encode())
                        print(f"ntff profile: {n} file(s) -> {output_dir}",
                              file=sys.stderr)

                hook = _hook
        except OSError:
            pass

    mod = types.ModuleType("antenv.axon_hooks")
    mod.get_axon_ntff_profile_hook = lambda: hook
    mod.set_axon_ntff_profile_hook = lambda h: None
    sys.modules["antenv.axon_hooks"] = mod


_install_ntff_hook_shim()

# Problem shapes (hardcoded per contract).
D, H, HD, FF, L, SYM, B = 256, 8, 32, 256, 3, 64, 32768
NCORES = 8
BC = B // NCORES          # 4096 rows per core
P = 128                   # SBUF partitions
NT = BC // P              # 32 tiles per core
F32 = mybir.dt.float32
I32 = mybir.dt.int32
BF16 = mybir.dt.bfloat16
AF = mybir.ActivationFunctionType
OP = mybir.AluOpType
EPS = 1e-5
SCALE = 1.0 / math.sqrt(HD)
# rsqrt bit-trick magic for input vh = v/2 (y0 ~ rsqrt(2*vh) = rsqrt(v))
RSQRT_MAGIC = 0x5F3759DF - 0x00400000


def _ln_stats(nc, pools, x_ap, ngroups):
    """rstd/-mu*rstd [P,ngroups] via bn_stats + DVE bit-trick rsqrt (no Sqrt LUT).

    x_ap: [P, ngroups, gsize] (or [P, gsize] if ngroups==1).
    """
    work = pools["work"]
    stats = work.tile([P, ngroups, 6], F32, tag="ln_stats")
    if ngroups == 1:
        nc.vector.bn_stats(out=stats[:, 0, :], in_=x_ap)
    else:
        for g in range(ngroups):
            nc.vector.bn_stats(out=stats[:, g, :], in_=x_ap[:, g, :])
    mv = work.tile([P, ngroups, 2], F32, tag="ln_mv")
    for g in range(ngroups):
        nc.vector.bn_aggr(out=mv[:, g, :], in_=stats[:, g, :])
    # vh = (var + eps) / 2
    vh = work.tile([P, ngroups], F32, tag="ln_vh")
    nc.vector.tensor_scalar(out=vh, in0=mv[:, :, 1], scalar1=EPS, scalar2=0.5,
                            op0=OP.add, op1=OP.mult)
    # y0 = bitcast(MAGIC - (bitcast(vh) >> 1))
    sh = work.tile([P, ngroups], I32, tag="ln_sh")
    nc.vector.tensor_scalar(out=sh, in0=vh.bitcast(I32), scalar1=1,
                            scalar2=None, op0=OP.logical_shift_right)
    y = work.tile([P, ngroups], F32, tag="ln_y")
    nc.vector.tensor_scalar(out=y.bitcast(I32), in0=sh, scalar1=-1,
                            scalar2=RSQRT_MAGIC, op0=OP.mult, op1=OP.add)
    # 3 Newton steps: y <- y * (1.5 - vh * y^2)
    t2 = work.tile([P, ngroups], F32, tag="ln_t2")
    w = work.tile([P, ngroups], F32, tag="ln_w")
    for _ in range(3):
        nc.vector.tensor_tensor(t2, y, y, OP.mult)
        nc.vector.tensor_tensor(t2, t2, vh, OP.mult)
        nc.vector.tensor_scalar(out=w, in0=t2, scalar1=-1.0, scalar2=1.5,
                                op0=OP.mult, op1=OP.add)
        nc.vector.tensor_tensor(y, y, w, OP.mult)
    # nmr = -(mu * rstd)
    nmr = work.tile([P, ngroups], F32, tag="ln_nmr")
    nc.vector.scalar_tensor_tensor(
        out=nmr, in0=mv[:, :, 0], scalar=-1.0, in1=y, op0=OP.mult, op1=OP.mult)
    return y, nmr


def _ln_apply(nc, pools, out_ap, x_ap, rstd, nmr, ngroups):
    """out[:,g,:] = x[:,g,:] * rstd[:,g] + nmr[:,g] (per-partition scalars)."""
    for g in range(ngroups):
        nc.vector.tensor_scalar(
            out=out_ap[:, g, :], in0=x_ap[:, g, :],
            scalar1=rstd[:, g:g + 1], scalar2=nmr[:, g:g + 1],
            op0=OP.mult, op1=OP.add)


def _transpose_to_lhst(nc, pools, src_ap, nchunks, tag):
    """PE-transpose src_ap [P, nchunks*128] -> SBUF lhsT [128, nchunks, 128]."""
    tp = pools["tpsum"]
    lhst = pools["lhst"].tile([P, nchunks, P], BF16, tag=tag)
    for c0 in range(0, nchunks, 4):
        cn = min(4, nchunks - c0)
        pt = tp.tile([P, 4, P], BF16, tag="tpsum")
        for c in range(cn):
            nc.tensor.transpose(
                pt[:, c, :], src_ap[:, (c0 + c) * P:(c0 + c + 1) * P], pools["identb"]
            )
        nc.scalar.copy(out=lhst[:, c0:c0 + cn, :], in_=pt[:, :cn, :])
    return lhst


def build_kernel(nc):
    """Trace the full forward pass for one core (BC rows)."""
    ge = nc.dram_tensor("ge", [BC, D], F32, kind="ExternalInput").ap()
    pe = nc.dram_tensor("pe", [BC, D], F32, kind="ExternalInput").ap()
    pp = nc.dram_tensor("pp", [BC, D], F32, kind="ExternalInput").ap()
    sf = nc.dram_tensor("sf", [BC, SYM], F32, kind="ExternalInput").ap()
    symw = nc.dram_tensor("symw", [P, D], F32, kind="ExternalInput").ap()
    wqkv = nc.dram_tensor("wqkv", [L, 2, P, 3 * D], F32, kind="ExternalInput").ap()
    wo = nc.dram_tensor("wo", [L, 2, P, D], F32, kind="ExternalInput").ap()
    w1 = nc.dram_tensor("w1", [L, 2, P, FF], F32, kind="ExternalInput").ap()
    w2 = nc.dram_tensor("w2", [L, 2, P, D], F32, kind="ExternalInput").ap()
    vecs = nc.dram_tensor("vecs", [9, D], F32, kind="ExternalInput").ap()
    out = nc.dram_tensor("out", [BC, D], F32, kind="ExternalOutput").ap()

    with ExitStack() as ctx:
        tc = ctx.enter_context(tile.TileContext(nc))
        singles = ctx.enter_context(tc.tile_pool(name="singles", bufs=1))
        work = ctx.enter_context(tc.tile_pool(name="work", bufs=3))
        xpool = ctx.enter_context(tc.tile_pool(name="xpool", bufs=2))
        qkvpool = ctx.enter_context(tc.tile_pool(name="qkvpool", bufs=2))
        lhstp = ctx.enter_context(tc.tile_pool(name="lhst", bufs=2))
        tpsum = ctx.enter_context(tc.tile_pool(name="tpsum", bufs=2, space="PSUM"))
        mmpsum = ctx.enter_context(tc.tile_pool(name="mmpsum", bufs=2, space="PSUM"))
        dpsum = ctx.enter_context(tc.tile_pool(name="dpsum", bufs=1, space="PSUM"))
        opool = ctx.enter_context(tc.tile_pool(name="opool", bufs=2))
        attw = ctx.enter_context(tc.tile_pool(name="attw", bufs=2))

        identb = singles.tile([P, P], BF16)
        make_identity(nc, identb)
        symw_sb = singles.tile([P, D], BF16)
        nc.gpsimd.dma_start(out=symw_sb, in_=symw)
        wqkv_sb = singles.tile([P, L, 2, 3 * D], BF16)
        nc.gpsimd.dma_start(out=wqkv_sb, in_=wqkv.transpose([2, 0, 1, 3]))
        wo_sb = singles.tile([P, L, 2, D], BF16)
        nc.gpsimd.dma_start(out=wo_sb, in_=wo.transpose([2, 0, 1, 3]))
        w1_sb = singles.tile([P, L, 2, FF], BF16)
        nc.gpsimd.dma_start(out=w1_sb, in_=w1.transpose([2, 0, 1, 3]))
        w2_sb = singles.tile([P, L, 2, D], BF16)
        nc.gpsimd.dma_start(out=w2_sb, in_=w2.transpose([2, 0, 1, 3]))
        vecs_sb = singles.tile([P, 9, D], F32)
        nc.sync.dma_start(out=vecs_sb, in_=vecs.partition_broadcast(P))

        pools = {"work": work, "tpsum": tpsum, "lhst": lhstp, "identb": identb}
        SYMG, SYMBT, TTE0, TTE1, TTE3 = 0, 1, 2, 3, 4
        FING, FINB, OUTG, OUTB = 5, 6, 7, 8

        for it in range(NT):
            row = it * P
            # ---- build x [P, 4, D] bf16 ----
            x = xpool.tile([P, 4, D], BF16, tag="x")
            ine = work.tile([P, 3, D], F32, tag="ine")
            nc.sync.dma_start(out=ine[:, 0, :], in_=ge[row:row + P, :])
            nc.sync.dma_start(out=ine[:, 1, :], in_=pe[row:row + P, :])
            nc.sync.dma_start(out=ine[:, 2, :], in_=pp[row:row + P, :])
            sft = work.tile([P, SYM], F32, tag="sft")
            nc.sync.dma_start(out=sft, in_=sf[row:row + P, :])

            nc.vector.tensor_add(x[:, 0, :], ine[:, 0, :], vecs_sb[:, TTE0, :])
            nc.vector.tensor_add(x[:, 1, :], ine[:, 1, :], vecs_sb[:, TTE1, :])
            nc.vector.tensor_add(x[:, 3, :], ine[:, 2, :], vecs_sb[:, TTE3, :])

            # sym branch: LN(sf @ symW) * g + (b + tte2)
            sftp = work.tile([P, P], BF16, tag="sftp")
            nc.vector.memset(sftp[:, SYM:], 0.0)
            nc.vector.tensor_copy(out=sftp[:, :SYM], in_=sft)
            spsum_t = tpsum.tile([P, 4, P], BF16, tag="tpsum", name="spsum")
            spsum = spsum_t[:, 0, :]
            nc.tensor.transpose(spsum, sftp, identb)
            slhst = work.tile([P, P], BF16, tag="slhst")
            nc.scalar.copy(out=slhst, in_=spsum)
            zsym_t = dpsum.tile([P, 4, D], F32, tag="mm_d4", name="zsym")
            zsym = zsym_t[:, 0, :]
            nc.tensor.matmul(zsym, slhst, symw_sb, start=True, stop=True)
            rstd, nmr = _ln_stats(nc, pools, zsym, 1)
            zn = work.tile([P, D], BF16, tag="zn")
            nc.vector.tensor_scalar(out=zn, in0=zsym, scalar1=rstd[:, 0:1],
                                    scalar2=nmr[:, 0:1], op0=OP.mult, op1=OP.add)
            nc.vector.tensor_tensor(x[:, 2, :], zn, vecs_sb[:, SYMG, :], OP.mult)
            nc.vector.tensor_add(x[:, 2, :], x[:, 2, :], vecs_sb[:, SYMBT, :])

            # ---- transformer layers ----
            for l in range(L):
                with nc.allow_low_precision(reason="bf16 transformer math"):
                    # LN1 (gains folded into wqkv)
                    rstd, nmr = _ln_stats(nc, pools, x, 4)
                    t = work.tile([P, 4, D], BF16, tag="t_ln")
                    _ln_apply(nc, pools, t, x, rstd, nmr, 4)
                    lhst = _transpose_to_lhst(
                        nc, pools, t.rearrange("p i d -> p (i d)"), 8, "lhst")
                    # qkv = t @ wqkv (activation-stationary)
                    qk_sb = qkvpool.tile([P, 4, 2 * D], BF16, tag="qk")
                    v_sb = qkvpool.tile([P, 4, D], BF16, tag="v")
                    for i in range(4):
                        mp = mmpsum.tile([P, 2, 512], F32, tag="mm_qkv")
                        for c in range(2):
                            nc.tensor.matmul(mp[:, 0, :], lhst[:, 2 * i + c, :],
                                             wqkv_sb[:, l, c, 0:512],
                                             start=(c == 0), stop=(c == 1))
                        for c in range(2):
                            nc.tensor.matmul(mp[:, 1, 0:D], lhst[:, 2 * i + c, :],
                                             wqkv_sb[:, l, c, 512:768],
                                             start=(c == 0), stop=(c == 1))
                        nc.scalar.copy(out=qk_sb[:, i, :], in_=mp[:, 0, :])
                        nc.scalar.copy(out=v_sb[:, i, :], in_=mp[:, 1, 0:D])

                    # ---- attention (bf16, contiguous) ----
                    qk4 = qk_sb.rearrange("p i (two d) -> p i two d", two=2)
                    q = qk4[:, :, 0, :]
                    k = qk4[:, :, 1, :]
                    prod = attw.tile([P, 4, 4, D], BF16, tag="att_prod")
                    qb = q[:, :, None, :].to_broadcast((P, 4, 4, D))
                    kb = k[:, None, :, :].to_broadcast((P, 4, 4, D))
                    nc.vector.tensor_tensor(prod, qb, kb, OP.mult)
                    # log-tree reduce over d=32 within each head
                    pr = prod.rearrange("p i j (h t f) -> p (i j) h t f", t=2, f=16)
                    s16 = attw.tile([P, 16, H, 16], BF16, tag="att_s16")
                    nc.vector.tensor_tensor(s16, pr[:, :, :, 0, :], pr[:, :, :, 1, :],
                                            OP.add)
                    r16 = s16.rearrange("p q h (t f) -> p q h t f", t=2)
                    s8 = attw.tile([P, 16, H, 8], BF16, tag="att_s8")
                    nc.vector.tensor_tensor(s8, r16[:, :, :, 0, :], r16[:, :, :, 1, :],
                                            OP.add)
                    r8 = s8.rearrange("p q h (t f) -> p q h t f", t=2)
                    s4 = attw.tile([P, 16, H, 4], BF16, tag="att_s4")
                    nc.vector.tensor_tensor(s4, r8[:, :, :, 0, :], r8[:, :, :, 1, :],
                                            OP.add)
                    r4 = s4.rearrange("p q h (t f) -> p q h t f", t=2)
                    s2 = attw.tile([P, 16, H, 2], BF16, tag="att_s2")
                    nc.vector.tensor_tensor(s2, r4[:, :, :, 0, :], r4[:, :, :, 1, :],
                                            OP.add)
                    sc = work.tile([P, 16, H], F32, tag="att_sc")
                    nc.vector.tensor_tensor(sc, s2[:, :, :, 0], s2[:, :, :, 1], OP.add)
                    # softmax over j via tanh: e^s = (1+t)/(1-t)
                    th = work.tile([P, 16, H], BF16, tag="att_th")
                    nc.scalar.activation(out=th, in_=sc, func=AF.Tanh,
                                         scale=SCALE * 0.5)
                    up = work.tile([P, 16, H], BF16, tag="att_up")
                    nc.vector.tensor_scalar(out=up, in0=th, scalar1=1.0,
                                            scalar2=None, op0=OP.add)
                    wm = work.tile([P, 16, H], BF16, tag="att_wm")
                    nc.vector.tensor_scalar(out=wm, in0=th, scalar1=-1.0,
                                            scalar2=1.0, op0=OP.mult, op1=OP.add)
                    uv = up.rearrange("p (i j) h -> p i j h", i=4)
                    wv = wm.rearrange("p (i j) h -> p i j h", i=4)
                    pA = work.tile([P, 4, H], BF16, tag="att_A")
                    pB = work.tile([P, 4, H], BF16, tag="att_B")
                    nc.vector.tensor_tensor(pA, wv[:, :, 0, :], wv[:, :, 1, :], OP.mult)
                    nc.vector.tensor_tensor(pB, wv[:, :, 2, :], wv[:, :, 3, :], OP.mult)
                    num = attw.tile([P, 4, 4, H], BF16, tag="att_num")
                    for j, (m0, m1) in enumerate(
                            [(wv[:, :, 1, :], pB), (wv[:, :, 0, :], pB),
                             (pA, wv[:, :, 3, :]), (pA, wv[:, :, 2, :])]):
                        cj = work.tile([P, 4, H], BF16, tag="att_c")
                        nc.vector.tensor_tensor(cj, m0, m1, OP.mult)
                        nc.vector.tensor_tensor(num[:, :, j, :], uv[:, :, j, :], cj,
                                                OP.mult)
                    d1 = work.tile([P, 4, H], BF16, tag="att_d1")
                    nc.vector.tensor_tensor(d1, num[:, :, 0, :], num[:, :, 1, :],
                                            OP.add)
                    den = work.tile([P, 4, H], F32, tag="att_den")
                    nc.vector.tensor_tensor(den, num[:, :, 2, :], num[:, :, 3, :],
                                            OP.add)
                    nc.vector.tensor_tensor(den, den, d1, OP.add)
                    dinv = work.tile([P, 4, H], F32, tag="att_dinv")
                    nc.vector.reciprocal(out=dinv, in_=den)
                    prob = work.tile([P, 4, 4, H], BF16, tag="att_prob")
                    nc.vector.tensor_tensor(
                        prob, num, dinv[:, :, None, :].to_broadcast((P, 4, 4, H)),
                        OP.mult)
                    # expand prob over d on the Scalar engine, pv on Vector
                    ppx = attw.tile([P, 4, 4, H, HD], BF16, tag="att_pp")
                    nc.scalar.copy(
                        out=ppx,
                        in_=prob[:, :, :, :, None].to_broadcast((P, 4, 4, H, HD)))
                    pvp = attw.tile([P, 4, 4, D], BF16, tag="att_pvp")
                    vb = v_sb[:, None, :, :].to_broadcast((P, 4, 4, D))
                    nc.vector.tensor_tensor(
                        pvp, ppx.rearrange("p i j h d -> p i j (h d)"), vb, OP.mult)
                    pv2 = pvp.rearrange("p i (t u) d -> p i t (u d)", t=2)
                    o1 = attw.tile([P, 4, 2 * D], BF16, tag="att_o1")
                    nc.vector.tensor_tensor(o1, pv2[:, :, 0, :], pv2[:, :, 1, :],
                                            OP.add)
                    ov = o1.rearrange("p i (t d) -> p i t d", t=2)
                    o = opool.tile([P, 4, D], BF16, tag="att_o")
                    nc.vector.tensor_tensor(o, ov[:, :, 0, :], ov[:, :, 1, :], OP.add)

                    # ---- o @ Wo + residual ----
                    lhsto = _transpose_to_lhst(
                        nc, pools, o.rearrange("p i d -> p (i d)"), 8, "lhst")
                    wo_ps = dpsum.tile([P, 4, D], F32, tag="mm_d4")
                    for i in range(4):
                        for c in range(2):
                            nc.tensor.matmul(wo_ps[:, i, :], lhsto[:, 2 * i + c, :],
                                             wo_sb[:, l, c, :],
                                             start=(c == 0), stop=(c == 1))
                    woev = work.tile([P, 4, D], BF16, tag="res_ev")
                    nc.scalar.copy(out=woev, in_=wo_ps)
                    nc.vector.tensor_tensor(
                        x.rearrange("p i d -> p (i d)"),
                        x.rearrange("p i d -> p (i d)"),
                        woev.rearrange("p i d -> p (i d)"), OP.add)

                    # ---- FF block ----
                    rstd, nmr = _ln_stats(nc, pools, x, 4)
                    t2 = work.tile([P, 4, D], BF16, tag="t2_ln")
                    _ln_apply(nc, pools, t2, x, rstd, nmr, 4)
                    lhst2 = _transpose_to_lhst(
                        nc, pools, t2.rearrange("p i d -> p (i d)"), 8, "lhst")
                    w1_ps = dpsum.tile([P, 4, FF], F32, tag="mm_d4")
                    for i in range(4):
                        for c in range(2):
                            nc.tensor.matmul(w1_ps[:, i, :], lhst2[:, 2 * i + c, :],
                                             w1_sb[:, l, c, :],
                                             start=(c == 0), stop=(c == 1))
                    gl = work.tile([P, 4, FF], BF16, tag="gelu")
                    nc.scalar.activation(out=gl, in_=w1_ps, func=AF.Gelu)
                    lhstg = _transpose_to_lhst(
                        nc, pools, gl.rearrange("p i d -> p (i d)"), 8, "lhst")
                    w2_ps = dpsum.tile([P, 4, D], F32, tag="mm_d4")
                    for i in range(4):
                        for c in range(2):
                            nc.tensor.matmul(w2_ps[:, i, :], lhstg[:, 2 * i + c, :],
                                             w2_sb[:, l, c, :],
                                             start=(c == 0), stop=(c == 1))
                    w2ev = work.tile([P, 4, D], BF16, tag="res_ev")
                    nc.scalar.copy(out=w2ev, in_=w2_ps)
                    nc.vector.tensor_tensor(
                        x.rearrange("p i d -> p (i d)"),
                        x.rearrange("p i d -> p (i d)"),
                        w2ev.rearrange("p i d -> p (i d)"), OP.add)

            # ---- tail: final_ln, mean over tokens, out_ln ----
            rstd, nmr = _ln_stats(nc, pools, x, 4)
            xt = work.tile([P, 4, D], F32, tag="tail_xt")
            _ln_apply(nc, pools, xt, x, rstd, nmr, 4)
            s01 = work.tile([P, 2, D], F32, tag="tail_s2")
            nc.vector.tensor_add(s01[:, 0, :], xt[:, 0, :], xt[:, 1, :])
            nc.vector.tensor_add(s01[:, 1, :], xt[:, 2, :], xt[:, 3, :])
            u = work.tile([P, D], F32, tag="tail_u")
            nc.vector.tensor_add(u, s01[:, 0, :], s01[:, 1, :])
            # u = 0.25*u*final_g + final_b
            nc.vector.scalar_tensor_tensor(
                out=u, in0=u, scalar=0.25, in1=vecs_sb[:, FING, :],
                op0=OP.mult, op1=OP.mult)
            nc.vector.tensor_add(u, u, vecs_sb[:, FINB, :])
            rstd, nmr = _ln_stats(nc, pools, u[:, None, :], 1)
            un = work.tile([P, D], F32, tag="tail_un")
            nc.vector.tensor_scalar(out=un, in0=u, scalar1=rstd[:, 0:1],
                                    scalar2=nmr[:, 0:1], op0=OP.mult, op1=OP.add)
            res = opool.tile([P, D], F32, tag="res")
            nc.vector.tensor_tensor(res, un, vecs_sb[:, OUTG, :], OP.mult)
            nc.vector.tensor_add(res, res, vecs_sb[:, OUTB, :])
            nc.sync.dma_start(out=out[row:row + P, :], in_=res)

    return nc


def _fold_host(inputs):
    """Fold LN gains/biases into weights on the host. Returns weight arrays."""
    f = lambda k: np.asarray(inputs[k], dtype=np.float32)
    wqkv, bqkv = f("Wqkv"), f("bqkv")
    wo, bo = f("Wo"), f("bo")
    w1, b1 = f("W1"), f("b1")
    w2, b2 = f("W2"), f("b2")
    g1, b1n = f("ln1_g"), f("ln1_b")
    g2, b2n = f("ln2_g"), f("ln2_b")

    wqkv_f = np.empty_like(wqkv)
    bqkv_f = np.empty_like(bqkv)
    w1_f = np.empty_like(w1)
    b1_f = np.empty_like(b1)
    for l in range(L):
        wqkv_f[l] = g1[l][:, None] * wqkv[l]
        bqkv_f[l] = b1n[l] @ wqkv[l] + bqkv[l]
        w1_f[l] = g2[l][:, None] * w1[l]
        b1_f[l] = b2n[l] @ w1[l] + b1[l]

    symw = np.zeros((P, D), dtype=np.float32)
    symw[:SYM] = f("sym_W")
    symb = f("sym_b")

    vecs = np.zeros((9, D), dtype=np.float32)
    tte = f("token_type_emb")
    vecs[0] = f("sym_ln_g")
    vecs[1] = f("sym_ln_b") + tte[2]
    vecs[2] = tte[0]
    vecs[3] = tte[1]
    vecs[4] = tte[3]
    vecs[5] = f("final_ln_g")
    vecs[6] = f("final_ln_b")
    vecs[7] = f("out_ln_g")
    vecs[8] = f("out_ln_b")

    bmisc = np.stack([bo, b1_f, b2], axis=1)  # [L, 3, D]
    nz = any(np.any(a) for a in (bqkv_f, bmisc, symb))
    return dict(symw=symw, symb=symb, wqkv=wqkv_f, bqkv=bqkv_f, wo=wo, w1=w1_f,
                w2=w2, vecs=vecs, bmisc=bmisc, nonzero_bias=bool(nz))


_CACHE = {}


def _get_built():
    key = "k2"
    if key not in _CACHE:
        from concourse import bacc
        nc = bacc.Bacc("TRN2", target_bir_lowering=False, debug=False,
                       num_devices=NCORES)
        build_kernel(nc)
        nc.compile()
        _CACHE[key] = nc
    return _CACHE[key]


def _chunk_w(w):
    """[L, 256, M] -> [L, 2, 128, M]"""
    Lx, K, M = w.shape
    return np.ascontiguousarray(w.reshape(Lx, 2, P, M))


def kernel(**inputs):
    fold = _fold_host(inputs)
    if fold["nonzero_bias"]:
        raise NotImplementedError("nonzero biases not supported in this build")

    nc = _get_built()

    ge = np.asarray(inputs["global_emb"], dtype=np.float32)
    pe = np.asarray(inputs["pert_emb"], dtype=np.float32)
    pp = np.asarray(inputs["ppi_feat"], dtype=np.float32)
    sf = np.asarray(inputs["sym_feat"], dtype=np.float32)

    wq = _chunk_w(fold["wqkv"])
    wo = _chunk_w(fold["wo"])
    w1 = _chunk_w(fold["w1"])
    w2 = _chunk_w(fold["w2"])

    in_maps = []
    for c in range(NCORES):
        sl = slice(c * BC, (c + 1) * BC)
        in_maps.append({
            "ge": np.ascontiguousarray(ge[sl]),
            "pe": np.ascontiguousarray(pe[sl]),
            "pp": np.ascontiguousarray(pp[sl]),
            "sf": np.ascontiguousarray(sf[sl]),
            "symw": fold["symw"],
            "wqkv": wq, "wo": wo, "w1": w1, "w2": w2,
            "vecs": fold["vecs"],
        })

    res = run_bass_kernel_spmd(nc, in_maps, core_ids=list(range(NCORES)))
    global LAST_RESULT
    LAST_RESULT = res
    outs = [res.results[c]["out"] for c in range(NCORES)]
    return np.concatenate(outs, axis=0)


LAST_RESULT = None


if __name__ == "__main__":
    print("smoke build only")
    _get_built()
    print("built ok")


# revision 10
# speedup vs baseline: 1.1417x; 1.1417x over previous
"""Trainium2 Bass kernel for nn_CrossAttentionFusion (dense_transformer).

Strategy: pure data parallel over 8 NeuronCores (batch 32768 -> 4096/core).
Token-major layout: batch rows on SBUF partitions, 4 tokens x 256 features in
the free dimension.  Dense matmuls are activation-stationary bf16 (fp32 PSUM).
Attention (seq=4, 8 heads x 32) runs on the Vector engine in bf16 with
contiguous access patterns: pair-products at 2x, log-tree adds for the d
reduction (tensor_reduce is capped at 1x), softmax via the tanh identity
e^s=(1+t)/(1-t) so the Scalar engine only ever needs Tanh+Gelu (one ACT table
set - no LUT reloads).  LayerNorm rstd is computed on the Vector engine with
the inverse-sqrt bit trick + 3 Newton steps (no Sqrt table), applied via
tensor_scalar with per-partition [P,1] scale/shift.  Residual stream is bf16;
residual adds are single [P,1024] ops after a Scalar-engine PSUM evacuation.
"""

import contextlib
import ctypes
import math
import os
import sys
import types
from contextlib import ExitStack

import numpy as np

import concourse.bass as bass
import concourse.tile as tile
from concourse import mybir
from concourse.bass_utils import run_bass_kernel_spmd
from concourse.masks import make_identity


def _install_ntff_hook_shim():
    """Provide antenv.axon_hooks if the image lacks it, so trace=True works."""
    try:
        import antenv.axon_hooks  # noqa: F401
        return
    except ImportError:
        pass
    so_path = "/opt/axon/libaxon_pjrt.so"
    hook = None
    if os.path.exists(so_path):
        try:
            lib = ctypes.CDLL(so_path)
            if hasattr(lib, "axon_start_nrt_profile"):
                lib.axon_start_nrt_profile.argtypes = [
                    ctypes.POINTER(ctypes.c_int64), ctypes.c_size_t]
                lib.axon_start_nrt_profile.restype = ctypes.c_int64
                lib.axon_stop_nrt_profile.argtypes = [ctypes.c_char_p]
                lib.axon_stop_nrt_profile.restype = ctypes.c_int64

                @contextlib.contextmanager
                def _hook(output_dir, device_ids):
                    import jax
                    jax.devices()
                    if device_ids:
                        ids = (ctypes.c_int64 * len(device_ids))(*device_ids)
                        rc = lib.axon_start_nrt_profile(ids, len(device_ids))
                    else:
                        rc = lib.axon_start_nrt_profile(None, 0)
                    if rc != 0:
                        raise RuntimeError(f"axon_start_nrt_profile rc={rc}")
                    try:
                        yield
                    finally:
                        n = lib.axon_stop_nrt_profile(str(output_dir).encode())
                        print(f"ntff profile: {n} file(s) -> {output_dir}",
                              file=sys.stderr)

                hook = _hook
        except OSError:
            pass

    mod = types.ModuleType("antenv.axon_hooks")
    mod.get_axon_ntff_profile_hook = lambda: hook
    mod.set_axon_ntff_profile_hook = lambda h: None
    sys.modules["antenv.axon_hooks"] = mod


_install_ntff_hook_shim()

# Problem shapes (hardcoded per contract).
D, H, HD, FF, L, SYM, B = 256, 8, 32, 256, 3, 64, 32768
NCORES = 8
BC = B // NCORES          # 4096 rows per core
P = 128                   # SBUF partitions
NT = BC // P              # 32 tiles per core
F32 = mybir.dt.float32
I32 = mybir.dt.int32
BF16 = mybir.dt.bfloat16
AF = mybir.ActivationFunctionType
OP = mybir.AluOpType
EPS = 1e-5
SCALE = 1.0 / math.sqrt(HD)
# rsqrt bit-trick magic for input vh = v/2 (y0 ~ rsqrt(2*vh) = rsqrt(v))
RSQRT_MAGIC = 0x5F3759DF - 0x00400000


def _ln_stats(nc, pools, x_ap, ngroups):
    """rstd/-mu*rstd [P,ngroups]: fast sums (ts/ttr + accum) + bit-trick rsqrt.

    x_ap: [P, ngroups, 256] (or [P, 256] if ngroups==1).
    """
    work = pools["work"]
    stats = work.tile([P, ngroups, 6], F32, tag="ln_stats")
    if ngroups == 1:
        nc.vector.bn_stats(out=stats[:, 0, :], in_=x_ap)
    else:
        for g in range(ngroups):
            nc.vector.bn_stats(out=stats[:, g, :], in_=x_ap[:, g, :])
    mv = work.tile([P, ngroups, 2], F32, tag="ln_mv")
    for g in range(ngroups):
        nc.vector.bn_aggr(out=mv[:, g, :], in_=stats[:, g, :])
    mu = mv[:, :, 0]
    # vh = (var + eps) / 2
    vh = work.tile([P, ngroups], F32, tag="ln_vh")
    nc.vector.tensor_scalar(out=vh, in0=mv[:, :, 1], scalar1=EPS, scalar2=0.5,
                            op0=OP.add, op1=OP.mult)
    # y0 = bitcast(MAGIC - (bitcast(vh) >> 1))
    sh = work.tile([P, ngroups], I32, tag="ln_sh")
    nc.vector.tensor_scalar(out=sh, in0=vh.bitcast(I32), scalar1=1,
                            scalar2=None, op0=OP.logical_shift_right)
    y = work.tile([P, ngroups], F32, tag="ln_y")
    nc.vector.tensor_scalar(out=y.bitcast(I32), in0=sh, scalar1=-1,
                            scalar2=RSQRT_MAGIC, op0=OP.mult, op1=OP.add)
    # 2 Newton steps: y <- y * (1.5 - vh * y^2)
    t2 = work.tile([P, ngroups], F32, tag="ln_t2")
    w = work.tile([P, ngroups], F32, tag="ln_w")
    for _ in range(2):
        nc.vector.tensor_tensor(t2, y, y, OP.mult)
        nc.vector.tensor_tensor(t2, t2, vh, OP.mult)
        nc.vector.tensor_scalar(out=w, in0=t2, scalar1=-1.0, scalar2=1.5,
                                op0=OP.mult, op1=OP.add)
        nc.vector.tensor_tensor(y, y, w, OP.mult)
    # nmr = -(mu * rstd)
    nmr = work.tile([P, ngroups], F32, tag="ln_nmr")
    nc.vector.scalar_tensor_tensor(
        out=nmr, in0=mu, scalar=-1.0, in1=y, op0=OP.mult, op1=OP.mult)
    return y, nmr


def _ln_apply(nc, pools, out_ap, x_ap, rstd, nmr, ngroups):
    """out[:,g,:] = x[:,g,:]*rstd[:,g] + nmr[:,g] on the Scalar engine."""
    for g in range(ngroups):
        nc.scalar.activation(
            out=out_ap[:, g, :], in_=x_ap[:, g, :], func=AF.Identity,
            bias=nmr[:, g:g + 1], scale=rstd[:, g:g + 1])


def _transpose_to_lhst(nc, pools, src_ap, nchunks, tag):
    """PE-transpose src_ap [P, nchunks*128] -> SBUF lhsT [128, nchunks, 128]."""
    tp = pools["tpsum"]
    lhst = pools["lhst"].tile([P, nchunks, P], BF16, tag=tag)
    for c0 in range(0, nchunks, 4):
        cn = min(4, nchunks - c0)
        pt = tp.tile([P, 4, P], BF16, tag="tpsum")
        for c in range(cn):
            nc.tensor.transpose(
                pt[:, c, :], src_ap[:, (c0 + c) * P:(c0 + c + 1) * P], pools["identb"]
            )
        nc.scalar.copy(out=lhst[:, c0:c0 + cn, :], in_=pt[:, :cn, :])
    return lhst


def build_kernel(nc):
    """Trace the full forward pass for one core (BC rows)."""
    ge = nc.dram_tensor("ge", [BC, D], F32, kind="ExternalInput").ap()
    pe = nc.dram_tensor("pe", [BC, D], F32, kind="ExternalInput").ap()
    pp = nc.dram_tensor("pp", [BC, D], F32, kind="ExternalInput").ap()
    sf = nc.dram_tensor("sf", [BC, SYM], F32, kind="ExternalInput").ap()
    symw = nc.dram_tensor("symw", [P, D], F32, kind="ExternalInput").ap()
    wqkv = nc.dram_tensor("wqkv", [L, 2, P, 3 * D], F32, kind="ExternalInput").ap()
    wo = nc.dram_tensor("wo", [L, 2, P, D], F32, kind="ExternalInput").ap()
    w1 = nc.dram_tensor("w1", [L, 2, P, FF], F32, kind="ExternalInput").ap()
    w2 = nc.dram_tensor("w2", [L, 2, P, D], F32, kind="ExternalInput").ap()
    vecs = nc.dram_tensor("vecs", [9, D], F32, kind="ExternalInput").ap()
    out = nc.dram_tensor("out", [BC, D], F32, kind="ExternalOutput").ap()

    with ExitStack() as ctx:
        tc = ctx.enter_context(tile.TileContext(nc))
        singles = ctx.enter_context(tc.tile_pool(name="singles", bufs=1))
        work = ctx.enter_context(tc.tile_pool(name="work", bufs=3))
        xpool = ctx.enter_context(tc.tile_pool(name="xpool", bufs=2))
        qkvpool = ctx.enter_context(tc.tile_pool(name="qkvpool", bufs=2))
        lhstp = ctx.enter_context(tc.tile_pool(name="lhst", bufs=4))
        tpsum = ctx.enter_context(tc.tile_pool(name="tpsum", bufs=2, space="PSUM"))
        dpsum = ctx.enter_context(tc.tile_pool(name="dpsum", bufs=3, space="PSUM"))
        opool = ctx.enter_context(tc.tile_pool(name="opool", bufs=2))
        attw = ctx.enter_context(tc.tile_pool(name="attw", bufs=2))

        identb = singles.tile([P, P], BF16)
        make_identity(nc, identb)
        symw_sb = singles.tile([P, D], BF16)
        nc.gpsimd.dma_start(out=symw_sb, in_=symw)
        wqkv_sb = singles.tile([P, L, 2, 3 * D], BF16)
        nc.gpsimd.dma_start(out=wqkv_sb, in_=wqkv.transpose([2, 0, 1, 3]))
        wo_sb = singles.tile([P, L, 2, D], BF16)
        nc.gpsimd.dma_start(out=wo_sb, in_=wo.transpose([2, 0, 1, 3]))
        w1_sb = singles.tile([P, L, 2, FF], BF16)
        nc.gpsimd.dma_start(out=w1_sb, in_=w1.transpose([2, 0, 1, 3]))
        w2_sb = singles.tile([P, L, 2, D], BF16)
        nc.gpsimd.dma_start(out=w2_sb, in_=w2.transpose([2, 0, 1, 3]))
        vecs_sb = singles.tile([P, 9, D], F32)
        nc.sync.dma_start(out=vecs_sb, in_=vecs.partition_broadcast(P))

        pools = {"work": work, "tpsum": tpsum, "lhst": lhstp, "identb": identb}
        SYMG, SYMBT, TTE0, TTE1, TTE3 = 0, 1, 2, 3, 4
        FING, FINB, OUTG, OUTB = 5, 6, 7, 8

        NW = 2  # row-tiles interleaved per iteration (cross-engine overlap)

        def stage_build_x(row):
            x = xpool.tile([P, 4, D], BF16, tag="x")
            ine = work.tile([P, 3, D], F32, tag="ine")
            nc.sync.dma_start(out=ine[:, 0, :], in_=ge[row:row + P, :])
            nc.sync.dma_start(out=ine[:, 1, :], in_=pe[row:row + P, :])
            nc.sync.dma_start(out=ine[:, 2, :], in_=pp[row:row + P, :])
            sft = work.tile([P, SYM], F32, tag="sft")
            nc.sync.dma_start(out=sft, in_=sf[row:row + P, :])

            nc.vector.tensor_add(x[:, 0, :], ine[:, 0, :], vecs_sb[:, TTE0, :])
            nc.vector.tensor_add(x[:, 1, :], ine[:, 1, :], vecs_sb[:, TTE1, :])
            nc.vector.tensor_add(x[:, 3, :], ine[:, 2, :], vecs_sb[:, TTE3, :])

            # sym branch: LN(sf @ symW) * g + (b + tte2)
            sftp = work.tile([P, P], BF16, tag="sftp")
            nc.vector.memset(sftp[:, SYM:], 0.0)
            nc.vector.tensor_copy(out=sftp[:, :SYM], in_=sft)
            spsum_t = tpsum.tile([P, 4, P], BF16, tag="tpsum", name="spsum")
            spsum = spsum_t[:, 0, :]
            nc.tensor.transpose(spsum, sftp, identb)
            slhst = work.tile([P, P], BF16, tag="slhst")
            nc.scalar.copy(out=slhst, in_=spsum)
            zsym_t = dpsum.tile([P, 4, D], F32, tag="mm_d4", name="zsym")
            zsym = zsym_t[:, 0, :]
            nc.tensor.matmul(zsym, slhst, symw_sb, start=True, stop=True)
            rstd, nmr = _ln_stats(nc, pools, zsym, 1)
            zn = work.tile([P, D], BF16, tag="zn")
            nc.scalar.activation(out=zn, in_=zsym, func=AF.Identity,
                                 bias=nmr[:, 0:1], scale=rstd[:, 0:1])
            nc.vector.tensor_tensor(x[:, 2, :], zn, vecs_sb[:, SYMG, :], OP.mult)
            nc.vector.tensor_add(x[:, 2, :], x[:, 2, :], vecs_sb[:, SYMBT, :])
            return x

        def stage_ln_transpose(x, tag):
            rstd, nmr = _ln_stats(nc, pools, x, 4)
            t = work.tile([P, 4, D], BF16, tag=tag)
            _ln_apply(nc, pools, t, x, rstd, nmr, 4)
            return _transpose_to_lhst(
                nc, pools, t.rearrange("p i d -> p (i d)"), 8, "lhst")

        def stage_qkv(l, lhst):
            qk_sb = qkvpool.tile([P, 4, 2 * D], BF16, tag="qk")
            v_sb = qkvpool.tile([P, 4, D], BF16, tag="v")
            for i in range(4):
                mp = dpsum.tile([P, 4, D], F32, tag="mm_d4")
                qkp = mp.rearrange("p i d -> p (i d)")
                for c in range(2):
                    nc.tensor.matmul(qkp[:, 0:512], lhst[:, 2 * i + c, :],
                                     wqkv_sb[:, l, c, 0:512],
                                     start=(c == 0), stop=(c == 1))
                for c in range(2):
                    nc.tensor.matmul(mp[:, 2, :], lhst[:, 2 * i + c, :],
                                     wqkv_sb[:, l, c, 512:768],
                                     start=(c == 0), stop=(c == 1))
                nc.scalar.copy(out=qk_sb[:, i, :], in_=qkp[:, 0:512])
                nc.scalar.copy(out=v_sb[:, i, :], in_=mp[:, 2, :])
            return qk_sb, v_sb

        def stage_attention(qk_sb, v_sb):
            qk4 = qk_sb.rearrange("p i (two d) -> p i two d", two=2)
            q = qk4[:, :, 0, :]
            k = qk4[:, :, 1, :]
            prod = attw.tile([P, 4, 4, D], BF16, tag="att_prod")
            qb = q[:, :, None, :].to_broadcast((P, 4, 4, D))
            kb = k[:, None, :, :].to_broadcast((P, 4, 4, D))
            nc.vector.tensor_tensor(prod, qb, kb, OP.mult)
            # log-tree reduce over d=32 within each head
            pr = prod.rearrange("p i j (h t f) -> p (i j) h t f", t=2, f=16)
            s16 = attw.tile([P, 16, H, 16], BF16, tag="att_s16")
            nc.vector.tensor_tensor(s16, pr[:, :, :, 0, :], pr[:, :, :, 1, :], OP.add)
            r16 = s16.rearrange("p q h (t f) -> p q h t f", t=2)
            s8 = attw.tile([P, 16, H, 8], BF16, tag="att_s8")
            nc.vector.tensor_tensor(s8, r16[:, :, :, 0, :], r16[:, :, :, 1, :], OP.add)
            r8 = s8.rearrange("p q h (t f) -> p q h t f", t=2)
            s4 = attw.tile([P, 16, H, 4], BF16, tag="att_s4")
            nc.vector.tensor_tensor(s4, r8[:, :, :, 0, :], r8[:, :, :, 1, :], OP.add)
            r4 = s4.rearrange("p q h (t f) -> p q h t f", t=2)
            s2 = attw.tile([P, 16, H, 2], BF16, tag="att_s2")
            nc.vector.tensor_tensor(s2, r4[:, :, :, 0, :], r4[:, :, :, 1, :], OP.add)
            sc = work.tile([P, 16, H], F32, tag="att_sc")
            nc.vector.tensor_tensor(sc, s2[:, :, :, 0], s2[:, :, :, 1], OP.add)
            # softmax over j via tanh: e^s = (1+t)/(1-t)
            th = work.tile([P, 16, H], BF16, tag="att_th")
            nc.scalar.activation(out=th, in_=sc, func=AF.Tanh, scale=SCALE * 0.5)
            up = work.tile([P, 16, H], BF16, tag="att_up")
            nc.vector.tensor_scalar(out=up, in0=th, scalar1=1.0, scalar2=None,
                                    op0=OP.add)
            wm = work.tile([P, 16, H], BF16, tag="att_wm")
            nc.vector.tensor_scalar(out=wm, in0=th, scalar1=-1.0, scalar2=1.0,
                                    op0=OP.mult, op1=OP.add)
            uv = up.rearrange("p (i j) h -> p i j h", i=4)
            wv = wm.rearrange("p (i j) h -> p i j h", i=4)
            pA = work.tile([P, 4, H], BF16, tag="att_A")
            pB = work.tile([P, 4, H], BF16, tag="att_B")
            nc.vector.tensor_tensor(pA, wv[:, :, 0, :], wv[:, :, 1, :], OP.mult)
            nc.vector.tensor_tensor(pB, wv[:, :, 2, :], wv[:, :, 3, :], OP.mult)
            num = attw.tile([P, 4, 4, H], BF16, tag="att_num")
            for j, (m0, m1) in enumerate(
                    [(wv[:, :, 1, :], pB), (wv[:, :, 0, :], pB),
                     (pA, wv[:, :, 3, :]), (pA, wv[:, :, 2, :])]):
                cj = work.tile([P, 4, H], BF16, tag="att_c")
                nc.vector.tensor_tensor(cj, m0, m1, OP.mult)
                nc.vector.tensor_tensor(num[:, :, j, :], uv[:, :, j, :], cj, OP.mult)
            d1 = work.tile([P, 4, H], BF16, tag="att_d1")
            nc.vector.tensor_tensor(d1, num[:, :, 0, :], num[:, :, 1, :], OP.add)
            den = work.tile([P, 4, H], F32, tag="att_den")
            nc.vector.tensor_tensor(den, num[:, :, 2, :], num[:, :, 3, :], OP.add)
            nc.vector.tensor_tensor(den, den, d1, OP.add)
            dinv = work.tile([P, 4, H], F32, tag="att_dinv")
            nc.vector.reciprocal(out=dinv, in_=den)
            prob = work.tile([P, 4, 4, H], BF16, tag="att_prob")
            nc.vector.tensor_tensor(
                prob, num, dinv[:, :, None, :].to_broadcast((P, 4, 4, H)), OP.mult)
            # expand prob over d on GpSimd (idle engine), pv on Vector
            ppx = attw.tile([P, 4, 4, H, HD], BF16, tag="att_pp")
            nc.gpsimd.tensor_copy(
                out=ppx, in_=prob[:, :, :, :, None].to_broadcast((P, 4, 4, H, HD)))
            pvp = attw.tile([P, 4, 4, D], BF16, tag="att_pvp")
            vb = v_sb[:, None, :, :].to_broadcast((P, 4, 4, D))
            nc.vector.tensor_tensor(
                pvp, ppx.rearrange("p i j h d -> p i j (h d)"), vb, OP.mult)
            pv2 = pvp.rearrange("p i (t u) d -> p i t (u d)", t=2)
            o1 = attw.tile([P, 4, 2 * D], BF16, tag="att_o1")
            nc.vector.tensor_tensor(o1, pv2[:, :, 0, :], pv2[:, :, 1, :], OP.add)
            ov = o1.rearrange("p i (t d) -> p i t d", t=2)
            o = opool.tile([P, 4, D], BF16, tag="att_o")
            nc.vector.tensor_tensor(o, ov[:, :, 0, :], ov[:, :, 1, :], OP.add)
            return o

        def stage_mm_residual(w_sb, l, lh, x, gelu_out=None):
            """lh^T @ w -> psum; either gelu-> SBUF or evac+residual-add to x."""
            ps = dpsum.tile([P, 4, D], F32, tag="mm_d4")
            for i in range(4):
                for c in range(2):
                    nc.tensor.matmul(ps[:, i, :], lh[:, 2 * i + c, :],
                                     w_sb[:, l, c, :],
                                     start=(c == 0), stop=(c == 1))
            if gelu_out is not None:
                nc.scalar.activation(out=gelu_out, in_=ps, func=AF.Gelu)
                return gelu_out
            ev = work.tile([P, 4, D], BF16, tag="res_ev")
            nc.scalar.copy(out=ev, in_=ps)
            nc.vector.tensor_tensor(
                x.rearrange("p i d -> p (i d)"), x.rearrange("p i d -> p (i d)"),
                ev.rearrange("p i d -> p (i d)"), OP.add)

        def stage_tail(x, row):
            rstd, nmr = _ln_stats(nc, pools, x, 4)
            xt = work.tile([P, 4, D], F32, tag="tail_xt")
            _ln_apply(nc, pools, xt, x, rstd, nmr, 4)
            s01 = work.tile([P, 2, D], F32, tag="tail_s2")
            nc.vector.tensor_add(s01[:, 0, :], xt[:, 0, :], xt[:, 1, :])
            nc.vector.tensor_add(s01[:, 1, :], xt[:, 2, :], xt[:, 3, :])
            u = work.tile([P, D], F32, tag="tail_u")
            nc.vector.tensor_add(u, s01[:, 0, :], s01[:, 1, :])
            # u = 0.25*u*final_g + final_b
            nc.vector.scalar_tensor_tensor(
                out=u, in0=u, scalar=0.25, in1=vecs_sb[:, FING, :],
                op0=OP.mult, op1=OP.mult)
            nc.vector.tensor_add(u, u, vecs_sb[:, FINB, :])
            rstd, nmr = _ln_stats(nc, pools, u[:, None, :], 1)
            un = work.tile([P, D], F32, tag="tail_un")
            nc.vector.tensor_scalar(out=un, in0=u, scalar1=rstd[:, 0:1],
                                    scalar2=nmr[:, 0:1], op0=OP.mult, op1=OP.add)
            res = opool.tile([P, D], F32, tag="res")
            nc.vector.tensor_tensor(res, un, vecs_sb[:, OUTG, :], OP.mult)
            nc.vector.tensor_add(res, res, vecs_sb[:, OUTB, :])
            nc.sync.dma_start(out=out[row:row + P, :], in_=res)

        for it0 in range(0, NT, NW):
            rows = [(it0 + s) * P for s in range(NW)]
            with nc.allow_low_precision(reason="bf16 transformer math"):
                xs = [stage_build_x(r) for r in rows]
                for l in range(L):
                    lhs = [stage_ln_transpose(xs[s], "t_ln") for s in range(NW)]
                    qkv = [stage_qkv(l, lhs[s]) for s in range(NW)]
                    os_ = [stage_attention(*qkv[s]) for s in range(NW)]
                    lho = [_transpose_to_lhst(
                        nc, pools, os_[s].rearrange("p i d -> p (i d)"), 8, "lhst")
                        for s in range(NW)]
                    for s in range(NW):
                        stage_mm_residual(wo_sb, l, lho[s], xs[s])
                    lh2 = [stage_ln_transpose(xs[s], "t2_ln") for s in range(NW)]
                    gls = []
                    for s in range(NW):
                        gl = work.tile([P, 4, FF], BF16, tag="gelu")
                        stage_mm_residual(w1_sb, l, lh2[s], None, gelu_out=gl)
                        gls.append(gl)
                    lhg = [_transpose_to_lhst(
                        nc, pools, gls[s].rearrange("p i d -> p (i d)"), 8, "lhst")
                        for s in range(NW)]
                    for s in range(NW):
                        stage_mm_residual(w2_sb, l, lhg[s], xs[s])
                for s in range(NW):
                    stage_tail(xs[s], rows[s])

    return nc


def _fold_host(inputs):
    """Fold LN gains/biases into weights on the host. Returns weight arrays."""
    f = lambda k: np.asarray(inputs[k], dtype=np.float32)
    wqkv, bqkv = f("Wqkv"), f("bqkv")
    wo, bo = f("Wo"), f("bo")
    w1, b1 = f("W1"), f("b1")
    w2, b2 = f("W2"), f("b2")
    g1, b1n = f("ln1_g"), f("ln1_b")
    g2, b2n = f("ln2_g"), f("ln2_b")

    wqkv_f = np.empty_like(wqkv)
    bqkv_f = np.empty_like(bqkv)
    w1_f = np.empty_like(w1)
    b1_f = np.empty_like(b1)
    for l in range(L):
        wqkv_f[l] = g1[l][:, None] * wqkv[l]
        bqkv_f[l] = b1n[l] @ wqkv[l] + bqkv[l]
        w1_f[l] = g2[l][:, None] * w1[l]
        b1_f[l] = b2n[l] @ w1[l] + b1[l]

    symw = np.zeros((P, D), dtype=np.float32)
    symw[:SYM] = f("sym_W")
    symb = f("sym_b")

    vecs = np.zeros((9, D), dtype=np.float32)
    tte = f("token_type_emb")
    vecs[0] = f("sym_ln_g")
    vecs[1] = f("sym_ln_b") + tte[2]
    vecs[2] = tte[0]
    vecs[3] = tte[1]
    vecs[4] = tte[3]
    vecs[5] = f("final_ln_g")
    vecs[6] = f("final_ln_b")
    vecs[7] = f("out_ln_g")
    vecs[8] = f("out_ln_b")

    bmisc = np.stack([bo, b1_f, b2], axis=1)  # [L, 3, D]
    nz = any(np.any(a) for a in (bqkv_f, bmisc, symb))
    return dict(symw=symw, symb=symb, wqkv=wqkv_f, bqkv=bqkv_f, wo=wo, w1=w1_f,
                w2=w2, vecs=vecs, bmisc=bmisc, nonzero_bias=bool(nz))


_CACHE = {}


def _get_built():
    key = "k2"
    if key not in _CACHE:
        from concourse import bacc
        nc = bacc.Bacc("TRN2", target_bir_lowering=False, debug=False,
                       num_devices=NCORES)
        build_kernel(nc)
        nc.compile()
        _CACHE[key] = nc
    return _CACHE[key]


def _chunk_w(w):
    """[L, 256, M] -> [L, 2, 128, M]"""
    Lx, K, M = w.shape
    return np.ascontiguousarray(w.reshape(Lx, 2, P, M))


def kernel(**inputs):
    fold = _fold_host(inputs)
    if fold["nonzero_bias"]:
        raise NotImplementedError("nonzero biases not supported in this build")

    nc = _get_built()

    ge = np.asarray(inputs["global_emb"], dtype=np.float32)
    pe = np.asarray(inputs["pert_emb"], dtype=np.float32)
    pp = np.asarray(inputs["ppi_feat"], dtype=np.float32)
    sf = np.asarray(inputs["sym_feat"], dtype=np.float32)

    wq = _chunk_w(fold["wqkv"])
    wo = _chunk_w(fold["wo"])
    w1 = _chunk_w(fold["w1"])
    w2 = _chunk_w(fold["w2"])

    in_maps = []
    for c in range(NCORES):
        sl = slice(c * BC, (c + 1) * BC)
        in_maps.append({
            "ge": np.ascontiguousarray(ge[sl]),
            "pe": np.ascontiguousarray(pe[sl]),
            "pp": np.ascontiguousarray(pp[sl]),
            "sf": np.ascontiguousarray(sf[sl]),
            "symw": fold["symw"],
            "wqkv": wq, "wo": wo, "w1": w1, "w2": w2,
            "vecs": fold["vecs"],
        })

    res = run_bass_kernel_spmd(nc, in_maps, core_ids=list(range(NCORES)))
    global LAST_RESULT
    LAST_RESULT = res
    outs = [res.results[c]["out"] for c in range(NCORES)]
    return np.concatenate(outs, axis=0)


LAST_RESULT = None


if __name__ == "__main__":
    print("smoke build only")
    _get_built()
    print("built ok")


# revision 11
# speedup vs baseline: 1.5728x; 1.3776x over previous
"""Trainium2 Bass kernel for nn_CrossAttentionFusion (dense_transformer).

Strategy: pure data parallel over 8 NeuronCores (batch 32768 -> 4096/core).
Token-major layout: batch rows on SBUF partitions, 4 tokens x 256 features in
the free dimension.  Dense matmuls are activation-stationary bf16 (fp32 PSUM).
Attention (seq=4, 8 heads x 32) runs on the Vector engine in bf16 with
contiguous access patterns: pair-products at 2x, log-tree adds for the d
reduction (tensor_reduce is capped at 1x), softmax via the tanh identity
e^s=(1+t)/(1-t) so the Scalar engine only ever needs Tanh+Gelu (one ACT table
set - no LUT reloads).  LayerNorm rstd is computed on the Vector engine with
the inverse-sqrt bit trick + 3 Newton steps (no Sqrt table), applied via
tensor_scalar with per-partition [P,1] scale/shift.  Residual stream is bf16;
residual adds are single [P,1024] ops after a Scalar-engine PSUM evacuation.
"""

import contextlib
import ctypes
import math
import os
import sys
import types
from contextlib import ExitStack

import numpy as np

import concourse.bass as bass
import concourse.tile as tile
from concourse import mybir
from concourse.bass_utils import run_bass_kernel_spmd
from concourse.masks import make_identity


def _install_ntff_hook_shim():
    """Provide antenv.axon_hooks if the image lacks it, so trace=True works."""
    try:
        import antenv.axon_hooks  # noqa: F401
        return
    except ImportError:
        pass
    so_path = "/opt/axon/libaxon_pjrt.so"
    hook = None
    if os.path.exists(so_path):
        try:
            lib = ctypes.CDLL(so_path)
            if hasattr(lib, "axon_start_nrt_profile"):
                lib.axon_start_nrt_profile.argtypes = [
                    ctypes.POINTER(ctypes.c_int64), ctypes.c_size_t]
                lib.axon_start_nrt_profile.restype = ctypes.c_int64
                lib.axon_stop_nrt_profile.argtypes = [ctypes.c_char_p]
                lib.axon_stop_nrt_profile.restype = ctypes.c_int64

                @contextlib.contextmanager
                def _hook(output_dir, device_ids):
                    import jax
                    jax.devices()
                    if device_ids:
                        ids = (ctypes.c_int64 * len(device_ids))(*device_ids)
                        rc = lib.axon_start_nrt_profile(ids, len(device_ids))
                    else:
                        rc = lib.axon_start_nrt_profile(None, 0)
                    if rc != 0:
                        raise RuntimeError(f"axon_start_nrt_profile rc={rc}")
                    try:
                        yield
                    finally:
                        n = lib.axon_stop_nrt_profile(str(output_dir).encode())
                        print(f"ntff profile: {n} file(s) -> {output_dir}",
                              file=sys.stderr)

                hook = _hook
        except OSError:
            pass

    mod = types.ModuleType("antenv.axon_hooks")
    mod.get_axon_ntff_profile_hook = lambda: hook
    mod.set_axon_ntff_profile_hook = lambda h: None
    sys.modules["antenv.axon_hooks"] = mod


_install_ntff_hook_shim()

# Problem shapes (hardcoded per contract).
D, H, HD, FF, L, SYM, B = 256, 8, 32, 256, 3, 64, 32768
NCORES = 8
BC = B // NCORES          # 4096 rows per core
P = 128                   # SBUF partitions
NT = BC // P              # 32 tiles per core
F32 = mybir.dt.float32
I32 = mybir.dt.int32
BF16 = mybir.dt.bfloat16
AF = mybir.ActivationFunctionType
OP = mybir.AluOpType
EPS = 1e-5
SCALE = 1.0 / math.sqrt(HD)
# rsqrt bit-trick magic for input vh = v/2 (y0 ~ rsqrt(2*vh) = rsqrt(v))
RSQRT_MAGIC = 0x5F3759DF - 0x00400000


def _ln_stats(nc, pools, x_ap, ngroups):
    """rstd/-mu*rstd [P,ngroups]: fast sums (ts/ttr + accum) + bit-trick rsqrt.

    x_ap: [P, ngroups, 256] (or [P, 256] if ngroups==1).
    """
    work = pools["work"]
    stats = work.tile([P, ngroups, 6], F32, tag="ln_stats")
    if ngroups == 1:
        nc.vector.bn_stats(out=stats[:, 0, :], in_=x_ap)
    else:
        for g in range(ngroups):
            nc.vector.bn_stats(out=stats[:, g, :], in_=x_ap[:, g, :])
    mv = work.tile([P, ngroups, 2], F32, tag="ln_mv")
    for g in range(ngroups):
        nc.vector.bn_aggr(out=mv[:, g, :], in_=stats[:, g, :])
    mu = mv[:, :, 0]
    # vh = (var + eps) / 2
    vh = work.tile([P, ngroups], F32, tag="ln_vh")
    nc.vector.tensor_scalar(out=vh, in0=mv[:, :, 1], scalar1=EPS, scalar2=0.5,
                            op0=OP.add, op1=OP.mult)
    # y0 = bitcast(MAGIC - (bitcast(vh) >> 1))
    sh = work.tile([P, ngroups], I32, tag="ln_sh")
    nc.vector.tensor_scalar(out=sh, in0=vh.bitcast(I32), scalar1=1,
                            scalar2=None, op0=OP.logical_shift_right)
    y = work.tile([P, ngroups], F32, tag="ln_y")
    nc.vector.tensor_scalar(out=y.bitcast(I32), in0=sh, scalar1=-1,
                            scalar2=RSQRT_MAGIC, op0=OP.mult, op1=OP.add)
    # 2 Newton steps: y <- y * (1.5 - vh * y^2)
    t2 = work.tile([P, ngroups], F32, tag="ln_t2")
    w = work.tile([P, ngroups], F32, tag="ln_w")
    for _ in range(2):
        nc.vector.tensor_tensor(t2, y, y, OP.mult)
        nc.vector.tensor_tensor(t2, t2, vh, OP.mult)
        nc.vector.tensor_scalar(out=w, in0=t2, scalar1=-1.0, scalar2=1.5,
                                op0=OP.mult, op1=OP.add)
        nc.vector.tensor_tensor(y, y, w, OP.mult)
    # nmr = -(mu * rstd)
    nmr = work.tile([P, ngroups], F32, tag="ln_nmr")
    nc.vector.scalar_tensor_tensor(
        out=nmr, in0=mu, scalar=-1.0, in1=y, op0=OP.mult, op1=OP.mult)
    return y, nmr


def _ln_apply(nc, pools, out_ap, x_ap, rstd, nmr, ngroups):
    """out[:,g,:] = x[:,g,:]*rstd[:,g] + nmr[:,g] on the Scalar engine."""
    for g in range(ngroups):
        nc.scalar.activation(
            out=out_ap[:, g, :], in_=x_ap[:, g, :], func=AF.Identity,
            bias=nmr[:, g:g + 1], scale=rstd[:, g:g + 1])


def _transpose_to_lhst(nc, pools, src_ap, nchunks, tag):
    """PE-transpose src_ap [P, nchunks*128] -> SBUF lhsT [128, nchunks, 128]."""
    tp = pools["tpsum"]
    lhst = pools["lhst"].tile([P, nchunks, P], BF16, tag=tag)
    for c0 in range(0, nchunks, 4):
        cn = min(4, nchunks - c0)
        pt = tp.tile([P, 4, P], BF16, tag="tpsum")
        for c in range(cn):
            nc.tensor.transpose(
                pt[:, c, :], src_ap[:, (c0 + c) * P:(c0 + c + 1) * P], pools["identb"]
            )
        nc.scalar.copy(out=lhst[:, c0:c0 + cn, :], in_=pt[:, :cn, :])
    return lhst


def build_kernel(nc):
    """Trace the full forward pass for one core (BC rows)."""
    ge = nc.dram_tensor("ge", [BC, D], F32, kind="ExternalInput").ap()
    pe = nc.dram_tensor("pe", [BC, D], F32, kind="ExternalInput").ap()
    pp = nc.dram_tensor("pp", [BC, D], F32, kind="ExternalInput").ap()
    sf = nc.dram_tensor("sf", [BC, SYM], F32, kind="ExternalInput").ap()
    symw = nc.dram_tensor("symw", [P, D], F32, kind="ExternalInput").ap()
    wqkv = nc.dram_tensor("wqkv", [L, 2, P, 3 * D], F32, kind="ExternalInput").ap()
    wo = nc.dram_tensor("wo", [L, 2, P, D], F32, kind="ExternalInput").ap()
    w1 = nc.dram_tensor("w1", [L, 2, P, FF], F32, kind="ExternalInput").ap()
    w2 = nc.dram_tensor("w2", [L, 2, P, D], F32, kind="ExternalInput").ap()
    vecs = nc.dram_tensor("vecs", [9, D], F32, kind="ExternalInput").ap()
    out = nc.dram_tensor("out", [BC, D], F32, kind="ExternalOutput").ap()

    with ExitStack() as ctx:
        tc = ctx.enter_context(tile.TileContext(nc))
        singles = ctx.enter_context(tc.tile_pool(name="singles", bufs=1))
        work = ctx.enter_context(tc.tile_pool(name="work", bufs=3))
        xpool = ctx.enter_context(tc.tile_pool(name="xpool", bufs=2))
        qkvpool = ctx.enter_context(tc.tile_pool(name="qkvpool", bufs=2))
        lhstp = ctx.enter_context(tc.tile_pool(name="lhst", bufs=4))
        tpsum = ctx.enter_context(tc.tile_pool(name="tpsum", bufs=2, space="PSUM"))
        dpsum = ctx.enter_context(tc.tile_pool(name="dpsum", bufs=3, space="PSUM"))
        opool = ctx.enter_context(tc.tile_pool(name="opool", bufs=2))
        attw = ctx.enter_context(tc.tile_pool(name="attw", bufs=2))

        identb = singles.tile([P, P], BF16)
        make_identity(nc, identb)
        symw_sb = singles.tile([P, D], BF16)
        nc.gpsimd.dma_start(out=symw_sb, in_=symw)
        wqkv_sb = singles.tile([P, L, 2, 3 * D], BF16)
        nc.gpsimd.dma_start(out=wqkv_sb, in_=wqkv.transpose([2, 0, 1, 3]))
        wo_sb = singles.tile([P, L, 2, D], BF16)
        nc.gpsimd.dma_start(out=wo_sb, in_=wo.transpose([2, 0, 1, 3]))
        w1_sb = singles.tile([P, L, 2, FF], BF16)
        nc.gpsimd.dma_start(out=w1_sb, in_=w1.transpose([2, 0, 1, 3]))
        w2_sb = singles.tile([P, L, 2, D], BF16)
        nc.gpsimd.dma_start(out=w2_sb, in_=w2.transpose([2, 0, 1, 3]))
        vecs_sb = singles.tile([P, 9, D], F32)
        nc.sync.dma_start(out=vecs_sb, in_=vecs.partition_broadcast(P))

        pools = {"work": work, "tpsum": tpsum, "lhst": lhstp, "identb": identb}
        SYMG, SYMBT, TTE0, TTE1, TTE3 = 0, 1, 2, 3, 4
        FING, FINB, OUTG, OUTB = 5, 6, 7, 8

        NW = 2  # row-tiles interleaved per iteration (cross-engine overlap)

        def stage_build_x(row):
            x = xpool.tile([P, 4, D], BF16, tag="x")
            ine = work.tile([P, 3, D], F32, tag="ine")
            nc.sync.dma_start(out=ine[:, 0, :], in_=ge[row:row + P, :])
            nc.sync.dma_start(out=ine[:, 1, :], in_=pe[row:row + P, :])
            nc.sync.dma_start(out=ine[:, 2, :], in_=pp[row:row + P, :])
            sft = work.tile([P, SYM], F32, tag="sft")
            nc.sync.dma_start(out=sft, in_=sf[row:row + P, :])

            nc.vector.tensor_add(x[:, 0, :], ine[:, 0, :], vecs_sb[:, TTE0, :])
            nc.vector.tensor_add(x[:, 1, :], ine[:, 1, :], vecs_sb[:, TTE1, :])
            nc.vector.tensor_add(x[:, 3, :], ine[:, 2, :], vecs_sb[:, TTE3, :])

            # sym branch: LN(sf @ symW) * g + (b + tte2)
            sftp = work.tile([P, P], BF16, tag="sftp")
            nc.vector.memset(sftp[:, SYM:], 0.0)
            nc.vector.tensor_copy(out=sftp[:, :SYM], in_=sft)
            spsum_t = tpsum.tile([P, 4, P], BF16, tag="tpsum", name="spsum")
            spsum = spsum_t[:, 0, :]
            nc.tensor.transpose(spsum, sftp, identb)
            slhst = work.tile([P, P], BF16, tag="slhst")
            nc.scalar.copy(out=slhst, in_=spsum)
            zsym_t = dpsum.tile([P, 4, D], F32, tag="mm_d4", name="zsym")
            zsym = zsym_t[:, 0, :]
            nc.tensor.matmul(zsym, slhst, symw_sb, start=True, stop=True)
            rstd, nmr = _ln_stats(nc, pools, zsym, 1)
            zn = work.tile([P, D], BF16, tag="zn")
            nc.scalar.activation(out=zn, in_=zsym, func=AF.Identity,
                                 bias=nmr[:, 0:1], scale=rstd[:, 0:1])
            nc.vector.tensor_tensor(x[:, 2, :], zn, vecs_sb[:, SYMG, :], OP.mult)
            nc.vector.tensor_add(x[:, 2, :], x[:, 2, :], vecs_sb[:, SYMBT, :])
            return x

        def stage_ln_transpose(x, tag):
            rstd, nmr = _ln_stats(nc, pools, x, 4)
            t = work.tile([P, 4, D], BF16, tag=tag)
            _ln_apply(nc, pools, t, x, rstd, nmr, 4)
            return _transpose_to_lhst(
                nc, pools, t.rearrange("p i d -> p (i d)"), 8, "lhst")

        def stage_qkv(l, lhst):
            qk_sb = qkvpool.tile([P, 4, 2 * D], BF16, tag="qk")
            v_sb = qkvpool.tile([P, 4, D], BF16, tag="v")
            for i in range(4):
                mp = dpsum.tile([P, 4, D], F32, tag="mm_d4")
                qkp = mp.rearrange("p i d -> p (i d)")
                for c in range(2):
                    nc.tensor.matmul(qkp[:, 0:512], lhst[:, 2 * i + c, :],
                                     wqkv_sb[:, l, c, 0:512],
                                     start=(c == 0), stop=(c == 1))
                for c in range(2):
                    nc.tensor.matmul(mp[:, 2, :], lhst[:, 2 * i + c, :],
                                     wqkv_sb[:, l, c, 512:768],
                                     start=(c == 0), stop=(c == 1))
                nc.scalar.copy(out=qk_sb[:, i, :], in_=qkp[:, 0:512])
                nc.scalar.copy(out=v_sb[:, i, :], in_=mp[:, 2, :])
            return qk_sb, v_sb

        def stage_attention(qk_sb, v_sb):
            qk4 = qk_sb.rearrange("p i (two d) -> p i two d", two=2)
            q = qk4[:, :, 0, :]
            k = qk4[:, :, 1, :]
            prod = attw.tile([P, 4, 4, D], BF16, tag="att_prod")
            qb = q[:, :, None, :].to_broadcast((P, 4, 4, D))
            kb = k[:, None, :, :].to_broadcast((P, 4, 4, D))
            nc.vector.tensor_tensor(prod, qb, kb, OP.mult)
            # log-tree reduce over d=32 within each head
            pr = prod.rearrange("p i j (h t f) -> p (i j) h t f", t=2, f=16)
            s16 = attw.tile([P, 16, H, 16], BF16, tag="att_s16")
            nc.vector.tensor_tensor(s16, pr[:, :, :, 0, :], pr[:, :, :, 1, :], OP.add)
            r16 = s16.rearrange("p q h (t f) -> p q h t f", t=2)
            s8 = attw.tile([P, 16, H, 8], BF16, tag="att_s8")
            nc.vector.tensor_tensor(s8, r16[:, :, :, 0, :], r16[:, :, :, 1, :], OP.add)
            r8 = s8.rearrange("p q h (t f) -> p q h t f", t=2)
            s4 = attw.tile([P, 16, H, 4], BF16, tag="att_s4")
            nc.vector.tensor_tensor(s4, r8[:, :, :, 0, :], r8[:, :, :, 1, :], OP.add)
            r4 = s4.rearrange("p q h (t f) -> p q h t f", t=2)
            s2 = attw.tile([P, 16, H, 2], BF16, tag="att_s2")
            nc.vector.tensor_tensor(s2, r4[:, :, :, 0, :], r4[:, :, :, 1, :], OP.add)
            sc = work.tile([P, 16, H], F32, tag="att_sc")
            nc.vector.tensor_tensor(sc, s2[:, :, :, 0], s2[:, :, :, 1], OP.add)
            # softmax over j via tanh: e^s = (1+t)/(1-t)
            th = work.tile([P, 16, H], BF16, tag="att_th")
            nc.scalar.activation(out=th, in_=sc, func=AF.Tanh, scale=SCALE * 0.5)
            up = work.tile([P, 16, H], BF16, tag="att_up")
            nc.vector.tensor_scalar(out=up, in0=th, scalar1=1.0, scalar2=None,
                                    op0=OP.add)
            wm = work.tile([P, 16, H], BF16, tag="att_wm")
            nc.vector.tensor_scalar(out=wm, in0=th, scalar1=-1.0, scalar2=1.0,
                                    op0=OP.mult, op1=OP.add)
            uv = up.rearrange("p (i j) h -> p i j h", i=4)
            wv = wm.rearrange("p (i j) h -> p i j h", i=4)
            pA = work.tile([P, 4, H], BF16, tag="att_A")
            pB = work.tile([P, 4, H], BF16, tag="att_B")
            nc.vector.tensor_tensor(pA, wv[:, :, 0, :], wv[:, :, 1, :], OP.mult)
            nc.vector.tensor_tensor(pB, wv[:, :, 2, :], wv[:, :, 3, :], OP.mult)
            num = attw.tile([P, 4, 4, H], BF16, tag="att_num")
            for j, (m0, m1) in enumerate(
                    [(wv[:, :, 1, :], pB), (wv[:, :, 0, :], pB),
                     (pA, wv[:, :, 3, :]), (pA, wv[:, :, 2, :])]):
                cj = work.tile([P, 4, H], BF16, tag="att_c")
                nc.vector.tensor_tensor(cj, m0, m1, OP.mult)
                nc.vector.tensor_tensor(num[:, :, j, :], uv[:, :, j, :], cj, OP.mult)
            d1 = work.tile([P, 4, H], BF16, tag="att_d1")
            nc.vector.tensor_tensor(d1, num[:, :, 0, :], num[:, :, 1, :], OP.add)
            den = work.tile([P, 4, H], F32, tag="att_den")
            nc.vector.tensor_tensor(den, num[:, :, 2, :], num[:, :, 3, :], OP.add)
            nc.vector.tensor_tensor(den, den, d1, OP.add)
            dinv = work.tile([P, 4, H], F32, tag="att_dinv")
            nc.vector.reciprocal(out=dinv, in_=den)
            prob = work.tile([P, 4, 4, H], BF16, tag="att_prob")
            nc.vector.tensor_tensor(
                prob, num, dinv[:, :, None, :].to_broadcast((P, 4, 4, H)), OP.mult)
            # expand prob over d on the Scalar engine, pv on Vector
            ppx = attw.tile([P, 4, 4, H, HD], BF16, tag="att_pp")
            nc.scalar.copy(
                out=ppx, in_=prob[:, :, :, :, None].to_broadcast((P, 4, 4, H, HD)))
            pvp = attw.tile([P, 4, 4, D], BF16, tag="att_pvp")
            vb = v_sb[:, None, :, :].to_broadcast((P, 4, 4, D))
            nc.vector.tensor_tensor(
                pvp, ppx.rearrange("p i j h d -> p i j (h d)"), vb, OP.mult)
            pv2 = pvp.rearrange("p i (t u) d -> p i t (u d)", t=2)
            o1 = attw.tile([P, 4, 2 * D], BF16, tag="att_o1")
            nc.vector.tensor_tensor(o1, pv2[:, :, 0, :], pv2[:, :, 1, :], OP.add)
            ov = o1.rearrange("p i (t d) -> p i t d", t=2)
            o = opool.tile([P, 4, D], BF16, tag="att_o")
            nc.vector.tensor_tensor(o, ov[:, :, 0, :], ov[:, :, 1, :], OP.add)
            return o

        def stage_mm_residual(w_sb, l, lh, x, gelu_out=None):
            """lh^T @ w -> psum; either gelu-> SBUF or evac+residual-add to x."""
            ps = dpsum.tile([P, 4, D], F32, tag="mm_d4")
            for i in range(4):
                for c in range(2):
                    nc.tensor.matmul(ps[:, i, :], lh[:, 2 * i + c, :],
                                     w_sb[:, l, c, :],
                                     start=(c == 0), stop=(c == 1))
            if gelu_out is not None:
                nc.scalar.activation(out=gelu_out, in_=ps, func=AF.Gelu)
                return gelu_out
            ev = work.tile([P, 4, D], BF16, tag="res_ev")
            nc.scalar.copy(out=ev, in_=ps)
            nc.vector.tensor_tensor(
                x.rearrange("p i d -> p (i d)"), x.rearrange("p i d -> p (i d)"),
                ev.rearrange("p i d -> p (i d)"), OP.add)

        def stage_tail(x, row):
            rstd, nmr = _ln_stats(nc, pools, x, 4)
            xt = work.tile([P, 4, D], F32, tag="tail_xt")
            _ln_apply(nc, pools, xt, x, rstd, nmr, 4)
            s01 = work.tile([P, 2, D], F32, tag="tail_s2")
            nc.vector.tensor_add(s01[:, 0, :], xt[:, 0, :], xt[:, 1, :])
            nc.vector.tensor_add(s01[:, 1, :], xt[:, 2, :], xt[:, 3, :])
            u = work.tile([P, D], F32, tag="tail_u")
            nc.vector.tensor_add(u, s01[:, 0, :], s01[:, 1, :])
            # u = 0.25*u*final_g + final_b
            nc.vector.scalar_tensor_tensor(
                out=u, in0=u, scalar=0.25, in1=vecs_sb[:, FING, :],
                op0=OP.mult, op1=OP.mult)
            nc.vector.tensor_add(u, u, vecs_sb[:, FINB, :])
            rstd, nmr = _ln_stats(nc, pools, u[:, None, :], 1)
            un = work.tile([P, D], F32, tag="tail_un")
            nc.vector.tensor_scalar(out=un, in0=u, scalar1=rstd[:, 0:1],
                                    scalar2=nmr[:, 0:1], op0=OP.mult, op1=OP.add)
            res = opool.tile([P, D], F32, tag="res")
            nc.vector.tensor_tensor(res, un, vecs_sb[:, OUTG, :], OP.mult)
            nc.vector.tensor_add(res, res, vecs_sb[:, OUTB, :])
            nc.sync.dma_start(out=out[row:row + P, :], in_=res)

        for it0 in range(0, NT, NW):
            rows = [(it0 + s) * P for s in range(NW)]
            with nc.allow_low_precision(reason="bf16 transformer math"):
                xs = [stage_build_x(r) for r in rows]
                for l in range(L):
                    lhs = [stage_ln_transpose(xs[s], "t_ln") for s in range(NW)]
                    qkv = [stage_qkv(l, lhs[s]) for s in range(NW)]
                    os_ = [stage_attention(*qkv[s]) for s in range(NW)]
                    lho = [_transpose_to_lhst(
                        nc, pools, os_[s].rearrange("p i d -> p (i d)"), 8, "lhst")
                        for s in range(NW)]
                    for s in range(NW):
                        stage_mm_residual(wo_sb, l, lho[s], xs[s])
                    lh2 = [stage_ln_transpose(xs[s], "t2_ln") for s in range(NW)]
                    gls = []
                    for s in range(NW):
                        gl = work.tile([P, 4, FF], BF16, tag="gelu")
                        stage_mm_residual(w1_sb, l, lh2[s], None, gelu_out=gl)
                        gls.append(gl)
                    lhg = [_transpose_to_lhst(
                        nc, pools, gls[s].rearrange("p i d -> p (i d)"), 8, "lhst")
                        for s in range(NW)]
                    for s in range(NW):
                        stage_mm_residual(w2_sb, l, lhg[s], xs[s])
                for s in range(NW):
                    stage_tail(xs[s], rows[s])

    return nc


def _fold_host(inputs):
    """Fold LN gains/biases into weights on the host. Returns weight arrays."""
    f = lambda k: np.asarray(inputs[k], dtype=np.float32)
    wqkv, bqkv = f("Wqkv"), f("bqkv")
    wo, bo = f("Wo"), f("bo")
    w1, b1 = f("W1"), f("b1")
    w2, b2 = f("W2"), f("b2")
    g1, b1n = f("ln1_g"), f("ln1_b")
    g2, b2n = f("ln2_g"), f("ln2_b")

    wqkv_f = np.empty_like(wqkv)
    bqkv_f = np.empty_like(bqkv)
    w1_f = np.empty_like(w1)
    b1_f = np.empty_like(b1)
    for l in range(L):
        wqkv_f[l] = g1[l][:, None] * wqkv[l]
        bqkv_f[l] = b1n[l] @ wqkv[l] + bqkv[l]
        w1_f[l] = g2[l][:, None] * w1[l]
        b1_f[l] = b2n[l] @ w1[l] + b1[l]

    symw = np.zeros((P, D), dtype=np.float32)
    symw[:SYM] = f("sym_W")
    symb = f("sym_b")

    vecs = np.zeros((9, D), dtype=np.float32)
    tte = f("token_type_emb")
    vecs[0] = f("sym_ln_g")
    vecs[1] = f("sym_ln_b") + tte[2]
    vecs[2] = tte[0]
    vecs[3] = tte[1]
    vecs[4] = tte[3]
    vecs[5] = f("final_ln_g")
    vecs[6] = f("final_ln_b")
    vecs[7] = f("out_ln_g")
    vecs[8] = f("out_ln_b")

    bmisc = np.stack([bo, b1_f, b2], axis=1)  # [L, 3, D]
    nz = any(np.any(a) for a in (bqkv_f, bmisc, symb))
    return dict(symw=symw, symb=symb, wqkv=wqkv_f, bqkv=bqkv_f, wo=wo, w1=w1_f,
                w2=w2, vecs=vecs, bmisc=bmisc, nonzero_bias=bool(nz))


_CACHE = {}


def _get_built():
    key = "k2"
    if key not in _CACHE:
        from concourse import bacc
        nc = bacc.Bacc("TRN2", target_bir_lowering=False, debug=False,
                       num_devices=NCORES)
        build_kernel(nc)
        nc.compile()
        _CACHE[key] = nc
    return _CACHE[key]


def _chunk_w(w):
    """[L, 256, M] -> [L, 2, 128, M]"""
    Lx, K, M = w.shape
    return np.ascontiguousarray(w.reshape(Lx, 2, P, M))


def kernel(**inputs):
    fold = _fold_host(inputs)
    if fold["nonzero_bias"]:
        raise NotImplementedError("nonzero biases not supported in this build")

    nc = _get_built()

    ge = np.asarray(inputs["global_emb"], dtype=np.float32)
    pe = np.asarray(inputs["pert_emb"], dtype=np.float32)
    pp = np.asarray(inputs["ppi_feat"], dtype=np.float32)
    sf = np.asarray(inputs["sym_feat"], dtype=np.float32)

    wq = _chunk_w(fold["wqkv"])
    wo = _chunk_w(fold["wo"])
    w1 = _chunk_w(fold["w1"])
    w2 = _chunk_w(fold["w2"])

    in_maps = []
    for c in range(NCORES):
        sl = slice(c * BC, (c + 1) * BC)
        in_maps.append({
            "ge": np.ascontiguousarray(ge[sl]),
            "pe": np.ascontiguousarray(pe[sl]),
            "pp": np.ascontiguousarray(pp[sl]),
            "sf": np.ascontiguousarray(sf[sl]),
            "symw": fold["symw"],
            "wqkv": wq, "wo": wo, "w1": w1, "w2": w2,
            "vecs": fold["vecs"],
        })

    res = run_bass_kernel_spmd(nc, in_maps, core_ids=list(range(NCORES)))
    global LAST_RESULT
    LAST_RESULT = res
    outs = [res.results[c]["out"] for c in range(NCORES)]
    return np.concatenate(outs, axis=0)


LAST_RESULT = None


if __name__ == "__main__":
    print("smoke build only")
    _get_built()
    print("built ok")
